# revision 8
# baseline (speedup 1.0000x reference)
"""BiasedMHA Trainium2 kernel: B=8 batches data-parallel across 8 NeuronCores.

Per core (one batch): fused attention with additive bias + boolean mask.
  out = softmax(Q@K^T*scale + bias, mask) @ V @ Wo^T + bo
Layout strategy: scores kept q-on-partitions so the (N,N,H) h-interleaved
bias DMAs contiguously; mask applied via PE matmul-accumulate (-1e30*I @ m);
PSUM evacuations fused into the bias-add (DVE) and exp (ACT); softmax
denominator via a ones-column appended to V.
"""

import sys

import numpy as np

for _p in ("/opt/trn_rl_repo",):
    if _p not in sys.path:
        sys.path.insert(0, _p)

import concourse.bass as bass  # noqa: E402
import concourse.mybir as mybir  # noqa: E402
import concourse.tile as tile  # noqa: E402
from concourse import bacc  # noqa: E402
from concourse.masks import make_identity  # noqa: E402

NN = 1024  # sequence length
F = 256  # feature dim
H = 8  # heads
D = F // H  # head dim = 32
P = 128  # partitions
NT = NN // P  # 8 q/seq tiles
FC = F // P  # 2 feature chunks
SCALE = D**-0.5
NEG = -1.0e30

F32 = mybir.dt.float32
BF16 = mybir.dt.bfloat16
U8 = mybir.dt.uint8
AF = mybir.ActivationFunctionType


def build_program():
    """Build the single-core program (one batch). Returns compiled Bacc."""
    nc = bacc.Bacc(
        "TRN2", target_bir_lowering=False, debug=False, num_devices=8
    )

    nd_dram = nc.dram_tensor("ndata", (NN, F), F32, kind="ExternalInput").ap()
    bias_dram = nc.dram_tensor(
        "attn_bias", (NN, NN, H), F32, kind="ExternalInput"
    ).ap()
    mask_dram = nc.dram_tensor(
        "attn_mask", (NN, NN), U8, kind="ExternalInput"
    ).ap()
    w_dram = {}
    b_dram = {}
    for w in ("q", "k", "v", "o"):
        w_dram[w] = nc.dram_tensor(f"W{w}", (F, F), F32, kind="ExternalInput").ap()
        b_dram[w] = nc.dram_tensor(f"b{w}", (F,), F32, kind="ExternalInput").ap()
    out_dram = nc.dram_tensor("out", (NN, F), F32, kind="ExternalOutput").ap()

    with tile.TileContext(nc) as tc:
        _emit(nc, tc, nd_dram, bias_dram, mask_dram, w_dram, b_dram, out_dram)

    nc.compile()
    return nc


def _emit(nc, tc, nd_dram, bias_dram, mask_dram, w_dram, b_dram, out_dram):
    from contextlib import ExitStack

    ctx = ExitStack()
    with ctx:
        const = ctx.enter_context(tc.tile_pool(name="const", bufs=1))
        wpool = ctx.enter_context(tc.tile_pool(name="wpool", bufs=1))
        biasp = ctx.enter_context(tc.tile_pool(name="biasp", bufs=2))
        mpool = ctx.enter_context(tc.tile_pool(name="mpool", bufs=2))
        spool = ctx.enter_context(tc.tile_pool(name="spool", bufs=3))
        epool = ctx.enter_context(tc.tile_pool(name="epool", bufs=3))
        small = ctx.enter_context(tc.tile_pool(name="small", bufs=2))
        psA = ctx.enter_context(tc.tile_pool(name="psA", bufs=2, space="PSUM"))
        psB = ctx.enter_context(tc.tile_pool(name="psB", bufs=1, space="PSUM"))
        psC = ctx.enter_context(tc.tile_pool(name="psC", bufs=1, space="PSUM"))

        # ---- constants ----
        i128b = const.tile([P, P], BF16, tag="i128b")
        make_identity(nc, i128b)
        i128f = const.tile([P, P], F32, tag="i128f")
        make_identity(nc, i128f)
        i33f = const.tile([33, 33], F32, tag="i33f")
        make_identity(nc, i33f)
        negI = const.tile([P, P], BF16, tag="negI")
        make_identity(nc, negI)
        nc.vector.tensor_scalar_mul(negI, negI, NEG)
        ones = const.tile([1, NN], BF16, tag="ones")
        nc.vector.memset(ones, 1.0)
        bb = {}
        for w in ("q", "k", "v", "o"):
            bf = const.tile([1, F], F32, tag=f"b{w}f")
            nc.sync.dma_start(out=bf, in_=b_dram[w][None, :])
            bh = const.tile([1, F], BF16, tag=f"b{w}h")
            nc.vector.tensor_copy(bh, bf)
            bb[w] = bh

        # ---- ndata and its transpose ----
        nd_sb = wpool.tile([P, NT, F], F32, tag="nd")
        nc.sync.dma_start(out=nd_sb, in_=nd_dram.rearrange("(t p) f -> p t f", p=P))
        nT = wpool.tile([P, FC, NN], BF16, tag="nT")
        for fc in range(FC):
            pst = psA.tile([P, NN], F32, tag="A")
            for t in range(NT):
                nc.tensor.transpose(
                    pst[:, t * P : (t + 1) * P],
                    nd_sb[:, t, fc * P : (fc + 1) * P],
                    i128f,
                )
            nc.scalar.copy(nT[:, fc, :], pst)

        # ---- weight transposes: WT[p, fic, fo] = W[fo, fic*128+p] ----
        wT = {}
        for w in ("q", "k", "v", "o"):
            wtmp = wpool.tile([P, FC, F], F32, tag="wtmp")
            nc.sync.dma_start(
                out=wtmp, in_=w_dram[w].rearrange("(c p) f -> p c f", p=P)
            )
            wt = wpool.tile([P, FC, F], BF16, tag=f"w{w}T")
            for fic in range(FC):
                psw = psB.tile([P, F], F32, tag="B")
                for foc in range(FC):
                    nc.tensor.transpose(
                        psw[:, foc * P : (foc + 1) * P],
                        wtmp[:, foc, fic * P : (fic + 1) * P],
                        i128f,
                    )
                nc.scalar.copy(wt[:, fic, :], psw)
            wT[w] = wt

        # ---- QT / KT head-major on partitions 0-31: (32, h, q), Q pre-scaled ----
        qt = wpool.tile([D, H, NN], BF16, tag="qt")
        kt = wpool.tile([D, H, NN], BF16, tag="kt")
        for name, dst, scl in (("q", qt, SCALE), ("k", kt, 1.0)):
            for h in range(H):
                ps = psA.tile([D, NN], F32, tag="A")
                for qh in range(2):
                    sl = slice(qh * 512, (qh + 1) * 512)
                    for fic in range(FC):
                        nc.tensor.matmul(
                            ps[:, sl],
                            lhsT=wT[name][:, fic, h * D : (h + 1) * D],
                            rhs=nT[:, fic, sl],
                            start=(fic == 0),
                            stop=False,
                        )
                    nc.tensor.matmul(
                        ps[:, sl],
                        lhsT=bb[name][:, h * D : (h + 1) * D],
                        rhs=ones[:, sl],
                        start=False,
                        stop=True,
                    )
                if scl == 1.0:
                    nc.scalar.copy(dst[:, h, :], ps)
                else:
                    nc.scalar.mul(dst[:, h, :], ps, scl)

        # ---- V' = [V_h | 1] per head: (seq-par tiles, h*33+d free) ----
        vp = wpool.tile([P, NT, H * 33], BF16, tag="vp")
        nc.vector.memset(vp, 1.0)
        for t in range(NT):
            psv = psB.tile([P, F], F32, tag="B")
            for fic in range(FC):
                nc.tensor.matmul(
                    psv,
                    lhsT=nT[:, fic, t * P : (t + 1) * P],
                    rhs=wT["v"][:, fic, :],
                    start=(fic == 0),
                    stop=False,
                )
            nc.tensor.matmul(
                psv, lhsT=ones[:, :P], rhs=bb["v"], start=False, stop=True
            )
            nc.scalar.copy(
                vp[:, t].rearrange("p (h dd) -> p h dd", dd=33)[:, :, :D],
                psv.rearrange("p (h dd) -> p h dd", dd=D),
            )

        # ---- main attention loop ----
        for t in range(NT):
            bias_t = biasp.tile([P, NN * H], F32, tag="bias")
            nc.sync.dma_start(
                out=bias_t,
                in_=bias_dram.rearrange("(t p) k h -> t p (k h)", p=P)[t],
            )
            m_u8 = mpool.tile([P, NN], U8, tag="mu8")
            nc.sync.dma_start(out=m_u8, in_=mask_dram[t * P : (t + 1) * P, :])
            m01 = mpool.tile([P, NN], BF16, tag="m01")
            nc.gpsimd.tensor_copy(m01, m_u8)
            nc.gpsimd.memset(m01[:, 0:1], 0.0)

            psc = psC.tile([33, H * P], F32, tag="C")
            for h in range(H):
                psa = psA.tile([P, NN], F32, tag="A")
                for kh in range(2):
                    sl = slice(kh * 512, (kh + 1) * 512)
                    nc.tensor.matmul(
                        psa[:, sl],
                        lhsT=qt[:, h, t * P : (t + 1) * P],
                        rhs=kt[:, h, sl],
                        start=True,
                        stop=False,
                    )
                    nc.tensor.matmul(
                        psa[:, sl],
                        lhsT=negI,
                        rhs=m01[:, sl],
                        start=False,
                        stop=True,
                    )
                sP = spool.tile([P, NN], BF16, tag="sP")
                nc.vector.tensor_add(
                    sP, psa, bias_t.rearrange("p (k h) -> p k h", h=H)[:, :, h]
                )
                # transpose via regular matmul (FWL + warm clock beat
                # is_transpose mode's fixed latency): psb = sP_chunk.T @ I
                psb = psB.tile([P, NN], F32, tag="B")
                for kc in range(NT):
                    nc.tensor.matmul(
                        psb[:, kc * P : (kc + 1) * P],
                        lhsT=sP[:, kc * P : (kc + 1) * P],
                        rhs=i128b,
                        start=True,
                        stop=True,
                    )
                eT = epool.tile([P, NN], BF16, tag="eT")
                nc.scalar.activation(eT, psb, AF.Exp)
                for kc in range(NT):
                    nc.tensor.matmul(
                        psc[:, h * P : (h + 1) * P],
                        lhsT=vp[:, kc, h * 33 : (h + 1) * 33],
                        rhs=eT[:, kc * P : (kc + 1) * P],
                        start=(kc == 0),
                        stop=(kc == NT - 1),
                    )

            # epilogue: divide by denominator, output projection
            oT = small.tile([33, H * P], F32, tag="oT")
            nc.scalar.copy(oT, psc)
            psd = psB.tile([P, H * 33], F32, tag="B")
            for h in range(H):
                nc.tensor.transpose(
                    psd[:, h * 33 : (h + 1) * 33],
                    oT[:, h * P : (h + 1) * P],
                    i33f,
                )
            den = small.tile([P, H], F32, tag="den")
            nc.vector.tensor_copy(
                den, psd.rearrange("p (h dd) -> p h dd", dd=33)[:, :, D]
            )
            rec = small.tile([P, H], F32, tag="rec")
            nc.vector.reciprocal(rec, den)
            a_sb = small.tile([P, F], BF16, tag="a_sb")
            for h in range(H):
                nc.scalar.activation(
                    a_sb[:, h * D : (h + 1) * D],
                    psd[:, h * 33 : h * 33 + D],
                    AF.Copy,
                    scale=rec[:, h : h + 1],
                )
            pst2 = psB.tile([P, F], BF16, tag="B")
            for fcc in range(FC):
                nc.tensor.transpose(
                    pst2[:, fcc * P : (fcc + 1) * P],
                    a_sb[:, fcc * P : (fcc + 1) * P],
                    i128b,
                )
            aT = small.tile([P, F], BF16, tag="aT")
            nc.scalar.copy(aT, pst2)
            psy = psA.tile([P, F], F32, tag="A")
            for fcc in range(FC):
                nc.tensor.matmul(
                    psy,
                    lhsT=aT[:, fcc * P : (fcc + 1) * P],
                    rhs=wT["o"][:, fcc, :],
                    start=(fcc == 0),
                    stop=False,
                )
            nc.tensor.matmul(
                psy, lhsT=ones[:, :P], rhs=bb["o"], start=False, stop=True
            )
            y_sb = small.tile([P, F], F32, tag="y")
            nc.scalar.copy(y_sb, psy)
            nc.sync.dma_start(out=out_dram[t * P : (t + 1) * P, :], in_=y_sb)


_CACHE = {}


def _make_in_maps(inputs):
    nd = np.asarray(inputs["ndata"], np.float32)
    ab = np.asarray(inputs["attn_bias"], np.float32)
    am = np.asarray(inputs["attn_mask"]).astype(np.uint8)
    ws = {
        f"W{w}": np.asarray(inputs[f"W{w}"], np.float32) for w in ("q", "k", "v", "o")
    }
    bs = {
        f"b{w}": np.asarray(inputs[f"b{w}"], np.float32) for w in ("q", "k", "v", "o")
    }
    in_maps = []
    for b in range(nd.shape[0]):
        m = {"ndata": nd[b], "attn_bias": ab[b], "attn_mask": am[b]}
        m.update(ws)
        m.update(bs)
        in_maps.append(m)
    return in_maps


def _get_nc():
    if "nc" not in _CACHE:
        _CACHE["nc"] = build_program()
    return _CACHE["nc"]


def _ensure_ntff_hook():
    """Shim antenv.axon_hooks (absent in this image) so trace=True works."""
    import types

    try:
        from antenv.axon_hooks import get_axon_ntff_profile_hook  # noqa: F401

        return
    except ImportError:
        pass
    import antenv

    mod = types.ModuleType("antenv.axon_hooks")
    _h = [None]
    mod.set_axon_ntff_profile_hook = lambda h: _h.__setitem__(0, h)
    mod.get_axon_ntff_profile_hook = lambda: _h[0]
    sys.modules["antenv.axon_hooks"] = mod
    antenv.axon_hooks = mod
    from trn_agent_boot.trn_boot import _ntff_profile_via_ctypes

    mod.set_axon_ntff_profile_hook(
        _ntff_profile_via_ctypes("/opt/axon/libaxon_pjrt.so")
    )


def run(inputs, trace=False):
    """Run on hardware; returns (output (B,N,F) f32, exec_time_ns or None)."""
    from concourse import bass_utils

    if trace:
        _ensure_ntff_hook()
    nc = _get_nc()
    in_maps = _make_in_maps(inputs)
    res = bass_utils.run_bass_kernel_spmd(
        nc, in_maps, core_ids=list(range(len(in_maps))), trace=trace
    )
    out = np.stack([r["out"] for r in res.results]).astype(np.float32)
    return out, res.exec_time_ns


def kernel(**inputs):
    out, _ = run(inputs, trace=False)
    return out


# revision 9
# speedup vs baseline: 1.0057x; 1.0057x over previous
"""BiasedMHA Trainium2 kernel: B=8 batches data-parallel across 8 NeuronCores.

Per core (one batch): fused attention with additive bias + boolean mask.
  out = softmax(Q@K^T*scale + bias, mask) @ V @ Wo^T + bo

Architecture (v3, tuned for the chip-level PE power throttle):
- scores kept q-on-partitions so the (N,N,H) h-interleaved bias DMAs
  contiguously; mask applied on PE via -1e30*I @ m accumulation into PSUM
- DVE adds bias straight from PSUM (fused evacuation), ACT computes exp with
  accum_out giving the softmax denominator for free; DVE folds 1/den into e
- the k-transpose of e runs on the DMA xbar (bf16, SBUF->SBUF, blocked
  (128,1024)->(128,8,128)), keeping it off the throttled TensorE
- attn@V is M=32 -> column-tiled 4 heads concurrently; its PSUM output is
  already A^T-chunk layout for the Wo projection
"""

import sys

import numpy as np

for _p in ("/opt/trn_rl_repo",):
    if _p not in sys.path:
        sys.path.insert(0, _p)

import concourse.bass as bass  # noqa: E402
import concourse.mybir as mybir  # noqa: E402
import concourse.tile as tile  # noqa: E402
from concourse import bacc  # noqa: E402
from concourse.masks import make_identity  # noqa: E402

NN = 1024  # sequence length
F = 256  # feature dim
H = 8  # heads
D = F // H  # head dim = 32
P = 128  # partitions
NT = NN // P  # 8 q/seq tiles
FC = F // P  # 2 feature chunks
SCALE = D**-0.5
NEG = -1.0e30

F32 = mybir.dt.float32
BF16 = mybir.dt.bfloat16
U8 = mybir.dt.uint8
AF = mybir.ActivationFunctionType


def build_program():
    """Build the single-core program (one batch). Returns compiled Bacc."""
    nc = bacc.Bacc(
        "TRN2", target_bir_lowering=False, debug=False, num_devices=8
    )

    nd_dram = nc.dram_tensor("ndata", (NN, F), F32, kind="ExternalInput").ap()
    bias_dram = nc.dram_tensor(
        "attn_bias", (NN, NN, H), F32, kind="ExternalInput"
    ).ap()
    mask_dram = nc.dram_tensor(
        "attn_mask", (NN, NN), U8, kind="ExternalInput"
    ).ap()
    w_dram = {}
    b_dram = {}
    for w in ("q", "k", "v", "o"):
        w_dram[w] = nc.dram_tensor(f"W{w}", (F, F), F32, kind="ExternalInput").ap()
        b_dram[w] = nc.dram_tensor(f"b{w}", (F,), F32, kind="ExternalInput").ap()
    out_dram = nc.dram_tensor("out", (NN, F), F32, kind="ExternalOutput").ap()

    with tile.TileContext(nc) as tc:
        _emit(nc, tc, nd_dram, bias_dram, mask_dram, w_dram, b_dram, out_dram)

    nc.compile()
    return nc


def _emit(nc, tc, nd_dram, bias_dram, mask_dram, w_dram, b_dram, out_dram):
    from contextlib import ExitStack

    ctx = ExitStack()
    with ctx:
        const = ctx.enter_context(tc.tile_pool(name="const", bufs=1))
        wpool = ctx.enter_context(tc.tile_pool(name="wpool", bufs=1))
        biasp = ctx.enter_context(tc.tile_pool(name="biasp", bufs=2))
        mpool = ctx.enter_context(tc.tile_pool(name="mpool", bufs=2))
        spool = ctx.enter_context(tc.tile_pool(name="spool", bufs=3))
        epool = ctx.enter_context(tc.tile_pool(name="epool", bufs=3))
        etp = ctx.enter_context(tc.tile_pool(name="etp", bufs=3))
        small = ctx.enter_context(tc.tile_pool(name="small", bufs=3))
        psA = ctx.enter_context(tc.tile_pool(name="psA", bufs=3, space="PSUM"))
        psC = ctx.enter_context(tc.tile_pool(name="psC", bufs=2, space="PSUM"))

        # ---- constants ----
        i128f = const.tile([P, P], F32, tag="i128f")
        make_identity(nc, i128f)
        negI = const.tile([P, P], BF16, tag="negI")
        make_identity(nc, negI)
        nc.vector.tensor_scalar_mul(negI, negI, NEG)
        ones = const.tile([1, NN], BF16, tag="ones")
        nc.vector.memset(ones, 1.0)
        bb = {}
        for w in ("q", "k", "v", "o"):
            bf = const.tile([1, F], F32, tag=f"b{w}f")
            nc.sync.dma_start(out=bf, in_=b_dram[w][None, :])
            bh = const.tile([1, F], BF16, tag=f"b{w}h")
            nc.vector.tensor_copy(bh, bf)
            bb[w] = bh

        # ---- ndata and its transpose ----
        nd_sb = wpool.tile([P, NT, F], F32, tag="nd")
        nc.sync.dma_start(out=nd_sb, in_=nd_dram.rearrange("(t p) f -> p t f", p=P))
        nT = wpool.tile([P, FC, NN], BF16, tag="nT")
        for fc in range(FC):
            pst = psA.tile([P, NN], F32, tag="A")
            for t in range(NT):
                nc.tensor.transpose(
                    pst[:, t * P : (t + 1) * P],
                    nd_sb[:, t, fc * P : (fc + 1) * P],
                    i128f,
                )
            nc.scalar.copy(nT[:, fc, :], pst)

        # ---- weight transposes: WT[p, fic, fo] = W[fo, fic*128+p] ----
        wT = {}
        for w in ("q", "k", "v", "o"):
            wtmp = wpool.tile([P, FC, F], F32, tag="wtmp")
            nc.sync.dma_start(
                out=wtmp, in_=w_dram[w].rearrange("(c p) f -> p c f", p=P)
            )
            wt = wpool.tile([P, FC, F], BF16, tag=f"w{w}T")
            for fic in range(FC):
                psw = psA.tile([P, F], F32, tag="A")
                for foc in range(FC):
                    nc.tensor.transpose(
                        psw[:, foc * P : (foc + 1) * P],
                        wtmp[:, foc, fic * P : (fic + 1) * P],
                        i128f,
                    )
                nc.scalar.copy(wt[:, fic, :], psw)
            wT[w] = wt

        # ---- QT / KT head-major on partitions 0-31: (32, h, q), Q pre-scaled ----
        qt = wpool.tile([D, H, NN], BF16, tag="qt")
        kt = wpool.tile([D, H, NN], BF16, tag="kt")
        for name, dst, scl in (("q", qt, SCALE), ("k", kt, 1.0)):
            for h in range(H):
                ps = psA.tile([D, NN], F32, tag="A")
                for qh in range(2):
                    sl = slice(qh * 512, (qh + 1) * 512)
                    for fic in range(FC):
                        nc.tensor.matmul(
                            ps[:, sl],
                            lhsT=wT[name][:, fic, h * D : (h + 1) * D],
                            rhs=nT[:, fic, sl],
                            start=(fic == 0),
                            stop=False,
                        )
                    nc.tensor.matmul(
                        ps[:, sl],
                        lhsT=bb[name][:, h * D : (h + 1) * D],
                        rhs=ones[:, sl],
                        start=False,
                        stop=True,
                    )
                if scl == 1.0:
                    nc.scalar.copy(dst[:, h, :], ps)
                else:
                    nc.scalar.mul(dst[:, h, :], ps, scl)

        # ---- V: (seq-par tiles, f free) ----
        vp = wpool.tile([P, NT, F], BF16, tag="vp")
        for t in range(NT):
            psv = psA.tile([P, F], F32, tag="A")
            for fic in range(FC):
                nc.tensor.matmul(
                    psv,
                    lhsT=nT[:, fic, t * P : (t + 1) * P],
                    rhs=wT["v"][:, fic, :],
                    start=(fic == 0),
                    stop=False,
                )
            nc.tensor.matmul(
                psv, lhsT=ones[:, :P], rhs=bb["v"], start=False, stop=True
            )
            nc.scalar.copy(vp[:, t, :], psv)

        # ---- main attention loop ----
        for t in range(NT):
            bias_t = biasp.tile([P, NN * H], F32, tag="bias")
            nc.sync.dma_start(
                out=bias_t,
                in_=bias_dram.rearrange("(t p) k h -> t p (k h)", p=P)[t],
            )
            m_u8 = mpool.tile([P, NN], U8, tag="mu8")
            nc.sync.dma_start(out=m_u8, in_=mask_dram[t * P : (t + 1) * P, :])
            m01 = mpool.tile([P, NN], BF16, tag="m01")
            nc.scalar.copy(m01, m_u8)
            nc.gpsimd.memset(m01[:, 0:1], 0.0)

            aT = small.tile([P, FC, P], BF16, tag="aT")
            psc = None
            for h in range(H):
                hg, j = h // 4, h % 4
                psa = psA.tile([P, NN], F32, tag="A")
                for kh in range(2):
                    sl = slice(kh * 512, (kh + 1) * 512)
                    nc.tensor.matmul(
                        psa[:, sl],
                        lhsT=qt[:, h, t * P : (t + 1) * P],
                        rhs=kt[:, h, sl],
                        start=True,
                        stop=False,
                    )
                    nc.tensor.matmul(
                        psa[:, sl],
                        lhsT=negI,
                        rhs=m01[:, sl],
                        start=False,
                        stop=True,
                    )
                sP = spool.tile([P, NN], BF16, tag="sP")
                nc.vector.tensor_add(
                    sP, psa, bias_t.rearrange("p (k h) -> p k h", h=H)[:, :, h]
                )
                den = small.tile([P, 1], F32, tag="den")
                e = epool.tile([P, NN], BF16, tag="e")
                nc.scalar.activation(e, sP, AF.Exp, accum_out=den)
                rec = small.tile([P, 1], F32, tag="rec")
                nc.vector.reciprocal(rec, den)
                nc.vector.tensor_scalar_mul(e, e, rec)
                eT = etp.tile([P, NT, P], BF16, tag="eT")
                nc.scalar.dma_start(out=eT, in_=e, transpose=True)
                if j == 0:
                    psc = psC.tile([P, P], F32, tag="C")
                for kc in range(NT):
                    nc.tensor.matmul(
                        psc[j * D : (j + 1) * D, :],
                        lhsT=vp[:, kc, h * D : (h + 1) * D],
                        rhs=eT[:, kc, :],
                        start=(kc == 0),
                        stop=(kc == NT - 1),
                        tile_position=(0, j * D),
                    )
                if j == 3:
                    nc.scalar.copy(aT[:, hg, :], psc)

            # output projection
            psy = psA.tile([P, F], F32, tag="A")
            for fcc in range(FC):
                nc.tensor.matmul(
                    psy,
                    lhsT=aT[:, fcc, :],
                    rhs=wT["o"][:, fcc, :],
                    start=(fcc == 0),
                    stop=False,
                )
            nc.tensor.matmul(
                psy, lhsT=ones[:, :P], rhs=bb["o"], start=False, stop=True
            )
            y_sb = small.tile([P, F], F32, tag="y")
            nc.scalar.copy(y_sb, psy)
            nc.sync.dma_start(out=out_dram[t * P : (t + 1) * P, :], in_=y_sb)


_CACHE = {}


def _make_in_maps(inputs):
    nd = np.asarray(inputs["ndata"], np.float32)
    ab = np.asarray(inputs["attn_bias"], np.float32)
    am = np.asarray(inputs["attn_mask"]).astype(np.uint8)
    ws = {
        f"W{w}": np.asarray(inputs[f"W{w}"], np.float32) for w in ("q", "k", "v", "o")
    }
    bs = {
        f"b{w}": np.asarray(inputs[f"b{w}"], np.float32) for w in ("q", "k", "v", "o")
    }
    in_maps = []
    for b in range(nd.shape[0]):
        m = {"ndata": nd[b], "attn_bias": ab[b], "attn_mask": am[b]}
        m.update(ws)
        m.update(bs)
        in_maps.append(m)
    return in_maps


def _get_nc():
    if "nc" not in _CACHE:
        _CACHE["nc"] = build_program()
    return _CACHE["nc"]


def _ensure_ntff_hook():
    """Shim antenv.axon_hooks (absent in this image) so trace=True works."""
    import types

    try:
        from antenv.axon_hooks import get_axon_ntff_profile_hook  # noqa: F401

        return
    except ImportError:
        pass
    import antenv

    mod = types.ModuleType("antenv.axon_hooks")
    _h = [None]
    mod.set_axon_ntff_profile_hook = lambda h: _h.__setitem__(0, h)
    mod.get_axon_ntff_profile_hook = lambda: _h[0]
    sys.modules["antenv.axon_hooks"] = mod
    antenv.axon_hooks = mod
    from trn_agent_boot.trn_boot import _ntff_profile_via_ctypes

    mod.set_axon_ntff_profile_hook(
        _ntff_profile_via_ctypes("/opt/axon/libaxon_pjrt.so")
    )


def run(inputs, trace=False):
    """Run on hardware; returns (output (B,N,F) f32, exec_time_ns or None)."""
    from concourse import bass_utils

    if trace:
        _ensure_ntff_hook()
    nc = _get_nc()
    in_maps = _make_in_maps(inputs)
    res = bass_utils.run_bass_kernel_spmd(
        nc, in_maps, core_ids=list(range(len(in_maps))), trace=trace
    )
    out = np.stack([r["out"] for r in res.results]).astype(np.float32)
    return out, res.exec_time_ns


def kernel(**inputs):
    out, _ = run(inputs, trace=False)
    return out


# revision 11
# speedup vs baseline: 1.1227x; 1.1163x over previous
"""BiasedMHA Trainium2 kernel: B=8 batches data-parallel across 8 NeuronCores.

Per core (one batch): fused attention with additive bias + boolean mask.
  out = softmax(Q@K^T*scale + bias, mask) @ V @ Wo^T + bo

Architecture (v3, tuned for the chip-level PE power throttle):
- scores kept q-on-partitions so the (N,N,H) h-interleaved bias DMAs
  contiguously; mask applied on PE via -1e30*I @ m accumulation into PSUM
- DVE adds bias straight from PSUM (fused evacuation), ACT computes exp with
  accum_out giving the softmax denominator for free; DVE folds 1/den into e
- the k-transpose of e runs on the DMA xbar (bf16, SBUF->SBUF, blocked
  (128,1024)->(128,8,128)), keeping it off the throttled TensorE
- attn@V is M=32 -> column-tiled 4 heads concurrently; its PSUM output is
  already A^T-chunk layout for the Wo projection
"""

import sys

import numpy as np

for _p in ("/opt/trn_rl_repo",):
    if _p not in sys.path:
        sys.path.insert(0, _p)

import concourse.bass as bass  # noqa: E402
import concourse.mybir as mybir  # noqa: E402
import concourse.tile as tile  # noqa: E402
from concourse import bacc  # noqa: E402
from concourse.masks import make_identity  # noqa: E402

NN = 1024  # sequence length
F = 256  # feature dim
H = 8  # heads
D = F // H  # head dim = 32
P = 128  # partitions
NT = NN // P  # 8 q/seq tiles
FC = F // P  # 2 feature chunks
SCALE = D**-0.5
NEG = -1.0e30

F32 = mybir.dt.float32
BF16 = mybir.dt.bfloat16
U8 = mybir.dt.uint8
AF = mybir.ActivationFunctionType


def build_program():
    """Build the single-core program (one batch). Returns compiled Bacc."""
    nc = bacc.Bacc(
        "TRN2", target_bir_lowering=False, debug=False, num_devices=8
    )

    nd_dram = nc.dram_tensor("ndata", (NN, F), F32, kind="ExternalInput").ap()
    bias_dram = nc.dram_tensor(
        "attn_bias", (NN, NN, H), F32, kind="ExternalInput"
    ).ap()
    mask_dram = nc.dram_tensor(
        "attn_mask", (NN, NN), U8, kind="ExternalInput"
    ).ap()
    w_dram = {}
    b_dram = {}
    for w in ("q", "k", "v", "o"):
        w_dram[w] = nc.dram_tensor(f"W{w}", (F, F), F32, kind="ExternalInput").ap()
        b_dram[w] = nc.dram_tensor(f"b{w}", (F,), F32, kind="ExternalInput").ap()
    out_dram = nc.dram_tensor("out", (NN, F), F32, kind="ExternalOutput").ap()

    with tile.TileContext(nc) as tc:
        _emit(nc, tc, nd_dram, bias_dram, mask_dram, w_dram, b_dram, out_dram)

    nc.compile()
    return nc


def _emit(nc, tc, nd_dram, bias_dram, mask_dram, w_dram, b_dram, out_dram):
    from contextlib import ExitStack

    ctx = ExitStack()
    with ctx:
        const = ctx.enter_context(tc.tile_pool(name="const", bufs=1))
        wpool = ctx.enter_context(tc.tile_pool(name="wpool", bufs=1))
        biasp = ctx.enter_context(tc.tile_pool(name="biasp", bufs=2))
        mpool = ctx.enter_context(tc.tile_pool(name="mpool", bufs=2))
        spool = ctx.enter_context(tc.tile_pool(name="spool", bufs=3))
        epool = ctx.enter_context(tc.tile_pool(name="epool", bufs=3))
        etp = ctx.enter_context(tc.tile_pool(name="etp", bufs=3))
        small = ctx.enter_context(tc.tile_pool(name="small", bufs=3))
        psA = ctx.enter_context(tc.tile_pool(name="psA", bufs=3, space="PSUM"))
        psC = ctx.enter_context(tc.tile_pool(name="psC", bufs=2, space="PSUM"))

        # ---- constants ----
        i128f = const.tile([P, P], F32, tag="i128f")
        make_identity(nc, i128f)
        negI = const.tile([P, P], BF16, tag="negI")
        make_identity(nc, negI)
        nc.vector.tensor_scalar_mul(negI, negI, NEG)
        ones = const.tile([1, NN], BF16, tag="ones")
        nc.vector.memset(ones, 1.0)
        bb = {}
        for w in ("q", "k", "v", "o"):
            bf = const.tile([1, F], F32, tag=f"b{w}f")
            nc.sync.dma_start(out=bf, in_=b_dram[w][None, :])
            bh = const.tile([1, F], BF16, tag=f"b{w}h")
            nc.vector.tensor_copy(bh, bf)
            bb[w] = bh

        # ---- ndata and its transpose ----
        nd_sb = wpool.tile([P, NT, F], F32, tag="nd")
        nc.sync.dma_start(out=nd_sb, in_=nd_dram.rearrange("(t p) f -> p t f", p=P))
        nT = wpool.tile([P, FC, NN], BF16, tag="nT")
        for fc in range(FC):
            pst = psA.tile([P, NN], F32, tag="A")
            for t in range(NT):
                nc.tensor.transpose(
                    pst[:, t * P : (t + 1) * P],
                    nd_sb[:, t, fc * P : (fc + 1) * P],
                    i128f,
                )
            nc.scalar.copy(nT[:, fc, :], pst)

        # ---- weight transposes: WT[p, fic, fo] = W[fo, fic*128+p] ----
        wT = {}
        for w in ("q", "k", "v", "o"):
            wtmp = wpool.tile([P, FC, F], F32, tag="wtmp")
            nc.sync.dma_start(
                out=wtmp, in_=w_dram[w].rearrange("(c p) f -> p c f", p=P)
            )
            wt = wpool.tile([P, FC, F], BF16, tag=f"w{w}T")
            for fic in range(FC):
                psw = psC.tile([P, F], F32, tag="C")
                for foc in range(FC):
                    nc.tensor.transpose(
                        psw[:, foc * P : (foc + 1) * P],
                        wtmp[:, foc, fic * P : (fic + 1) * P],
                        i128f,
                    )
                nc.scalar.copy(wt[:, fic, :], psw)
            wT[w] = wt

        # ---- QT / KT: head h at partitions 32*(h%4), plane h//4; Q pre-scaled ----
        qt = wpool.tile([P, H // 4, NN], BF16, tag="qt")
        kt = wpool.tile([P, H // 4, NN], BF16, tag="kt")
        for name, dst, scl in (("q", qt, SCALE), ("k", kt, 1.0)):
            for c in range(H // 4):
                ps = psA.tile([P, NN], F32, tag="A")
                for j in range(4):
                    h = c * 4 + j
                    rs = slice(j * D, (j + 1) * D)
                    for qh in range(2):
                        sl = slice(qh * 512, (qh + 1) * 512)
                        for fic in range(FC):
                            nc.tensor.matmul(
                                ps[rs, sl],
                                lhsT=wT[name][:, fic, h * D : (h + 1) * D],
                                rhs=nT[:, fic, sl],
                                start=(fic == 0),
                                stop=False,
                                tile_position=(0, j * D),
                            )
                        nc.tensor.matmul(
                            ps[rs, sl],
                            lhsT=bb[name][:, h * D : (h + 1) * D],
                            rhs=ones[:, sl],
                            start=False,
                            stop=True,
                            tile_position=(0, j * D),
                        )
                if scl == 1.0:
                    nc.scalar.copy(dst[:, c, :], ps)
                else:
                    nc.scalar.mul(dst[:, c, :], ps, scl)

        # ---- V: (seq-par tiles, f free) ----
        vp = wpool.tile([P, NT, F], BF16, tag="vp")
        for t in range(NT):
            psv = psC.tile([P, F], F32, tag="C")
            for fic in range(FC):
                nc.tensor.matmul(
                    psv,
                    lhsT=nT[:, fic, t * P : (t + 1) * P],
                    rhs=wT["v"][:, fic, :],
                    start=(fic == 0),
                    stop=False,
                )
            nc.tensor.matmul(
                psv, lhsT=ones[:, :P], rhs=bb["v"], start=False, stop=True
            )
            nc.scalar.copy(vp[:, t, :], psv)

        # ---- main attention loop (bias/mask prefetched one tile ahead) ----
        bias_tiles = {}
        mask_tiles = {}

        def load_t(tt):
            bias_tiles[tt] = biasp.tile(
                [P, NN * H], F32, tag="bias", name=f"bias_{tt}"
            )
            nc.sync.dma_start(
                out=bias_tiles[tt],
                in_=bias_dram.rearrange("(t p) k h -> t p (k h)", p=P)[tt],
            )
            mask_tiles[tt] = mpool.tile([P, NN], U8, tag="mu8", name=f"mu8_{tt}")
            nc.sync.dma_start(
                out=mask_tiles[tt], in_=mask_dram[tt * P : (tt + 1) * P, :]
            )

        load_t(0)
        for t in range(NT):
            if t + 1 < NT:
                load_t(t + 1)
            bias_t = bias_tiles.pop(t)
            m_u8 = mask_tiles.pop(t)
            m01 = mpool.tile([P, NN], BF16, tag="m01")
            nc.scalar.copy(m01, m_u8)
            nc.gpsimd.memset(m01[:, 0:1], 0.0)

            aT = small.tile([P, FC, P], BF16, tag="aT")
            psc = None
            for h in range(H):
                hg, j = h // 4, h % 4
                psa = psA.tile([P, NN], F32, tag="A")
                for kh in range(2):
                    sl = slice(kh * 512, (kh + 1) * 512)
                    nc.tensor.matmul(
                        psa[:, sl],
                        lhsT=qt[j * D : (j + 1) * D, hg, t * P : (t + 1) * P],
                        rhs=kt[j * D : (j + 1) * D, hg, sl],
                        start=True,
                        stop=False,
                        tile_position=(j * D, 0),
                    )
                    nc.tensor.matmul(
                        psa[:, sl],
                        lhsT=negI,
                        rhs=m01[:, sl],
                        start=False,
                        stop=True,
                    )
                sP = spool.tile([P, NN], BF16, tag="sP")
                nc.vector.tensor_add(
                    sP, psa, bias_t.rearrange("p (k h) -> p k h", h=H)[:, :, h]
                )
                den = small.tile([P, 1], F32, tag="den")
                e = epool.tile([P, NN], BF16, tag="e")
                nc.scalar.activation(e, sP, AF.Exp, accum_out=den)
                rec = small.tile([P, 1], F32, tag="rec")
                nc.vector.reciprocal(rec, den)
                nc.vector.tensor_scalar_mul(e, e, rec)
                eT = etp.tile([P, NT, P], BF16, tag="eT")
                nc.scalar.dma_start(out=eT, in_=e, transpose=True)
                if j == 0:
                    psc = psC.tile([P, P], F32, tag="C")
                for kc in range(NT):
                    nc.tensor.matmul(
                        psc[j * D : (j + 1) * D, :],
                        lhsT=vp[:, kc, h * D : (h + 1) * D],
                        rhs=eT[:, kc, :],
                        start=(kc == 0),
                        stop=(kc == NT - 1),
                        tile_position=(0, j * D),
                    )
                if j == 3:
                    nc.scalar.copy(aT[:, hg, :], psc)

            # output projection
            psy = psA.tile([P, F], F32, tag="A")
            for fcc in range(FC):
                nc.tensor.matmul(
                    psy,
                    lhsT=aT[:, fcc, :],
                    rhs=wT["o"][:, fcc, :],
                    start=(fcc == 0),
                    stop=False,
                )
            nc.tensor.matmul(
                psy, lhsT=ones[:, :P], rhs=bb["o"], start=False, stop=True
            )
            y_sb = small.tile([P, F], F32, tag="y")
            nc.scalar.copy(y_sb, psy)
            nc.sync.dma_start(out=out_dram[t * P : (t + 1) * P, :], in_=y_sb)


_CACHE = {}


def _make_in_maps(inputs):
    nd = np.asarray(inputs["ndata"], np.float32)
    ab = np.asarray(inputs["attn_bias"], np.float32)
    am = np.asarray(inputs["attn_mask"]).astype(np.uint8)
    ws = {
        f"W{w}": np.asarray(inputs[f"W{w}"], np.float32) for w in ("q", "k", "v", "o")
    }
    bs = {
        f"b{w}": np.asarray(inputs[f"b{w}"], np.float32) for w in ("q", "k", "v", "o")
    }
    in_maps = []
    for b in range(nd.shape[0]):
        m = {"ndata": nd[b], "attn_bias": ab[b], "attn_mask": am[b]}
        m.update(ws)
        m.update(bs)
        in_maps.append(m)
    return in_maps


def _get_nc():
    if "nc" not in _CACHE:
        _CACHE["nc"] = build_program()
    return _CACHE["nc"]


def _ensure_ntff_hook():
    """Shim antenv.axon_hooks (absent in this image) so trace=True works."""
    import types

    try:
        from antenv.axon_hooks import get_axon_ntff_profile_hook  # noqa: F401

        return
    except ImportError:
        pass
    import antenv

    mod = types.ModuleType("antenv.axon_hooks")
    _h = [None]
    mod.set_axon_ntff_profile_hook = lambda h: _h.__setitem__(0, h)
    mod.get_axon_ntff_profile_hook = lambda: _h[0]
    sys.modules["antenv.axon_hooks"] = mod
    antenv.axon_hooks = mod
    from trn_agent_boot.trn_boot import _ntff_profile_via_ctypes

    mod.set_axon_ntff_profile_hook(
        _ntff_profile_via_ctypes("/opt/axon/libaxon_pjrt.so")
    )


def run(inputs, trace=False):
    """Run on hardware; returns (output (B,N,F) f32, exec_time_ns or None)."""
    from concourse import bass_utils

    if trace:
        _ensure_ntff_hook()
    nc = _get_nc()
    in_maps = _make_in_maps(inputs)
    res = bass_utils.run_bass_kernel_spmd(
        nc, in_maps, core_ids=list(range(len(in_maps))), trace=trace
    )
    out = np.stack([r["out"] for r in res.results]).astype(np.float32)
    return out, res.exec_time_ns


def kernel(**inputs):
    out, _ = run(inputs, trace=False)
    return out
